# revision 36
# baseline (speedup 1.0000x reference)
"""Trainium2 Bass kernel for a dense transformer block (B=4, T=2048, C=1024, H=16).

Sharding (8 cores): core c handles batch b=c//2 and head-group hg=c%2
(8 heads). Each core computes LN1 + QKV + causal attention for its 8 heads
over the full T=2048, then a 2-core ReduceScatter exchanges attnT halves
within each (batch) pair so core c finishes proj + LN2 + FFN for its own
T-half (rows hg*1024 .. hg*1024+1024) with the full set of 16 heads.

Structure (v2):
- LN1/LN2 normalize on ACT (Identity with per-partition scale/bias), and the
  feature-major transposes go through the DMA XBAR (16-bit dma transpose)
  instead of PE+ACT evictions.
- Attention: scores in PSUM f32 (diagonal blocks subranged to columns
  >= 128j), exp on ACT -> ee bf16, causal mask applied post-exp as a {0,1}
  bf16 multiply on the [128,128] triangle block only, AV matmuls in bf16
  with ones-augmented V ([64 dims, 1] per head) so the softmax denominator
  falls out in row 64.  Normalization: DVE reciprocal + PE rank-1 broadcast
  + DVE multiply on SBUF-assembled operands.
- t4 iteration order (2,0,3,1) fires the first ReduceScatter mid-attention
  so proj for the first own-T-half overlaps the attention tail; LN2 + FFN
  follow per T-half.
- FFN streams W1 per half [128c x 512m] tiles (contiguous lines) and W2
  per (half, cp), all bf16.
"""

import sys
import numpy as np

for _p in ("/opt/trn_rl_repo",):
    if _p not in sys.path:
        sys.path.append(_p)

import concourse.bass as bass
import concourse.bacc as bacc
import concourse.tile as tile
import concourse.mybir as mybir

dt = mybir.dt
AF = mybir.ActivationFunctionType
ALU = mybir.AluOpType
F32 = dt.float32
F32R = dt.float32r
BF16 = dt.bfloat16

N_CORES = 8
B, T, C = 4, 2048, 1024
H, HS = 16, 64
HL = 8            # heads per core (local)
TH = T // 2       # t-half (rows per core for proj/FFN)
FF = 4 * C        # 4096
EPS = 1e-5

BF16_NP = dt.np(BF16)

_PROGRAM = None
NO_COLLECTIVE = False  # replace RS with local DMA (for TimelineSim)
DEBUG_DUMP = False

NT = T // 128          # 16 t-tiles (full T)
NTH = TH // 128        # 8 t-tiles (own half)
NC8 = C // 128         # 8 c-chunks
NPAIR = HL // 2        # 4 head pairs
NM = FF // 128         # 32 FFN m-blocks
T4_ORDER = (2, 0, 3, 1)

# ---- packed-input blob offsets (elements) ----
# All inputs ship as TWO dram tensors (one bf16, one f32): per-call dispatch
# cost through the axon tunnel scales with operand count (~36us/operand), so
# 13 separate tensors -> 2 blobs. Layout on host matches the exact order the
# device DMAs each block, so every transfer is contiguous per partition.
X_OFF = 0
WQ_OFF = X_OFF + T * C                      # 2_097_152
WK_OFF = WQ_OFF + C * HL * HS               # +524_288
WV_OFF = WK_OFF + C * HL * HS
WP_OFF = WV_OFF + C * HL * HS
W1_OFF = WP_OFF + C * C
W2_OFF = W1_OFF + C * FF
TRI_OFF = W2_OFF + FF * C
B2R_OFF = TRI_OFF + 128 * 2 * 128
ONES_OFF = B2R_OFF + C
BF_TOTAL = ONES_OFF + 128

XO_OFF = 0
B1C_OFF = XO_OFF + TH * C
SELC_OFF = B1C_OFF + 128 * NM
F32_TOTAL = SELC_OFF + 128 * 2


def _build_program():
    nc = bacc.Bacc(
        "TRN2",
        target_bir_lowering=False,
        debug=False,
        num_devices=N_CORES,
        enable_partition_id=True,
    )

    # ---- I/O (packed: see blob offset table above) ----
    blob_h = nc.dram_tensor("blob_h", [BF_TOTAL], BF16, kind="ExternalInput")
    blob_f = nc.dram_tensor("blob_f", [F32_TOTAL], F32, kind="ExternalInput")
    out = nc.dram_tensor("out_half", [TH, C], F32, kind="ExternalOutput")

    def bh(off, pattern, **axes):
        n = 1
        for v in axes.values():
            n *= v
        return blob_h[off : off + n].rearrange(pattern, **axes)

    def bf(off, pattern, **axes):
        n = 1
        for v in axes.values():
            n *= v
        return blob_f[off : off + n].rearrange(pattern, **axes)
    if DEBUG_DUMP:
        dbg_xT = nc.dram_tensor("dbg_xT", [128, NC8, T], BF16, kind="ExternalOutput")
        dbg_kT = nc.dram_tensor("dbg_kT", [128, T], BF16, kind="ExternalOutput")
        dbg_qT = nc.dram_tensor("dbg_qT", [128, T], BF16, kind="ExternalOutput")
        dbg_v = nc.dram_tensor("dbg_v", [128, HL, 65], BF16, kind="ExternalOutput")
        dbg_attnT = nc.dram_tensor("dbg_attnT", [NPAIR, 128, T], BF16, kind="ExternalOutput")
        dbg_attnP = nc.dram_tensor("dbg_attnP", [128, NPAIR, TH], BF16, kind="ExternalOutput")
        dbg_x2 = nc.dram_tensor("dbg_x2", [TH, C], F32, kind="ExternalOutput")

    with tile.TileContext(nc) as tc:
        from contextlib import ExitStack

        ctx = ExitStack()
        with ctx:
            # ---------------- pools ----------------
            consts = ctx.enter_context(tc.tile_pool(name="consts", bufs=1))
            ps_mm = ctx.enter_context(tc.tile_pool(name="ps_mm", bufs=2, space="PSUM"))
            ps_sc = ctx.enter_context(tc.tile_pool(name="ps_sc", bufs=2, space="PSUM"))
            ps_av = ctx.enter_context(tc.tile_pool(name="ps_av", bufs=1, space="PSUM"))
            dram = ctx.enter_context(tc.tile_pool(name="dram", bufs=1, space="DRAM"))

            work = ctx.enter_context(tc.tile_pool(name="work", bufs=2, side="left"))
            ln_pool = ctx.enter_context(tc.tile_pool(name="ln", bufs=6, side="left"))
            exp_pool = ctx.enter_context(tc.tile_pool(name="exp", bufs=3, side="left"))
            attn_sm = ctx.enter_context(
                tc.tile_pool(name="attn_sm", bufs=2, side="left")
            )
            # phase-scoped pools (strict LIFO per side):
            # right: poolA -> (close) -> wpP -> poolC -> (close both) -> w1p/w2p
            # left:  poolB -> (close) -> poolE -> poolF
            ctxA = ExitStack()   # xT + wq/wk/wv         — dies after v production
            ctxB = ExitStack()   # kT, qT, v             — dies after attention
            ctxW = ExitStack()   # wproj                 — dies after proj
            ctxC = ExitStack()   # attnT, attnP          — dies after proj
            poolA = ctxA.enter_context(tc.tile_pool(name="poolA", bufs=1, side="right"))
            poolB = ctxB.enter_context(tc.tile_pool(name="poolB", bufs=1, side="left"))

            # ---------------- constants ----------------
            tri = consts.tile([128, 2, 128], BF16, name="tri_t")
            nc.sync.dma_start(tri[:], bh(TRI_OFF, "(p k m) -> p k m", p=128, k=2, m=128))
            b1c = consts.tile([128, NM], F32, name="b1c_t")
            nc.sync.dma_start(b1c[:], bf(B1C_OFF, "(p m) -> p m", p=128, m=NM))
            b2r = consts.tile([1, C], BF16, name="b2r_t")
            nc.sync.dma_start(b2r[:], bh(B2R_OFF, "(a c) -> a c", a=1, c=C))
            ones_bf = consts.tile([1, 128], BF16, name="ones_bf_t")
            nc.sync.dma_start(ones_bf[:], bh(ONES_OFF, "(a c) -> a c", a=1, c=128))
            selc = consts.tile([128, 2], F32, name="selc_t")
            nc.sync.dma_start(selc[:], bf(SELC_OFF, "(p s) -> p s", p=128, s=2))
            eps_t = consts.tile([128, 1], F32, name="eps_t")
            nc.vector.memset(eps_t[:], EPS)
            neg1 = consts.tile([128, 1], F32, name="neg1_t")
            nc.vector.memset(neg1[:], -1.0)
            zero_c = consts.tile([128, 1], F32, name="zero_c")
            nc.vector.memset(zero_c[:], 0.0)

            # qkv weights, feature-major: [128, 4, 512] quads (one DMA each)
            wq_q = [poolA.tile([128, 4, HL * HS], BF16, name=f"wq4_{g}") for g in range(2)]
            wk_q = [poolA.tile([128, 4, HL * HS], BF16, name=f"wk4_{g}") for g in range(2)]
            wv_q = [poolA.tile([128, 4, HL * HS], BF16, name=f"wv4_{g}") for g in range(2)]
            for g in range(2):
                for wt_q, w_off in ((wq_q, WQ_OFF), (wk_q, WK_OFF), (wv_q, WV_OFF)):
                    nc.sync.dma_start(
                        wt_q[g][:],
                        bh(w_off + 512 * g * (HL * HS),
                           "(k p m) -> p k m", k=4, p=128, m=HL * HS),
                    )
            wq_t = [wq_q[i // 4][:, i % 4, :] for i in range(NC8)]
            wk_t = [wk_q[i // 4][:, i % 4, :] for i in range(NC8)]
            wv_t = [wv_q[i // 4][:, i % 4, :] for i in range(NC8)]

            def ln_norm(x_sb):
                """LayerNorm x_sb [128, C] f32 -> xh bf16 (stats on DVE,
                normalize on ACT)."""
                st = ln_pool.tile([128, 2, 6], F32, tag="st")
                nc.vector.bn_stats(st[:, 0, :], x_sb[:, 0:512])
                nc.vector.bn_stats(st[:, 1, :], x_sb[:, 512:1024])
                mv = ln_pool.tile([128, 2], F32, tag="mv")
                nc.vector.bn_aggr(mv[:], st[:])
                sd = ln_pool.tile([128, 1], F32, tag="sd")
                nc.scalar.activation(sd[:], mv[:, 1:2], AF.Sqrt, bias=eps_t[:])
                rs = ln_pool.tile([128, 1], F32, tag="rs")
                nc.vector.reciprocal_approx_fast(rs[:], sd[:])
                bb = ln_pool.tile([128, 1], F32, tag="bb")
                nc.vector.tensor_scalar(
                    bb[:], mv[:, 0:1], rs[:], neg1[:], ALU.mult, ALU.mult
                )
                xh = work.tile([128, C], BF16, tag="xh", bufs=3)
                with nc.allow_low_precision(reason="bf16 ln out"):
                    nc.scalar.activation(
                        xh[:], x_sb[:], AF.Identity, bias=bb[:], scale=rs[:]
                    )
                return xh

            def ln_transpose(dst3, xh, i):
                # xbar transpose: out[p, j, t] = xh[t, 128j + p]
                # issued on the sync queue: descriptor-gen costs ~1.3us of the
                # issuing engine, and ACT is the busy one
                nc.sync.dma_start(
                    dst3[:, :, 128 * i : 128 * i + 128], xh[:], transpose=True
                )

            # ---------------- phase A+B: LN1 -> xT, V + KQ interleaved ------
            # V for tile i needs only tile i's transpose, and KQ for t4-chunk
            # g needs tiles 4g..4g+3 — so PE work starts after ONE tile's LN
            # chain instead of after all sixteen.
            xT = poolA.tile([128, NC8, T], BF16, name="xT")  # x-hat transposed
            kT = [poolB.tile([128, T], BF16, name=f"kT_{p}") for p in range(NPAIR)]
            qT = [poolB.tile([128, T], BF16, name=f"qT_{p}") for p in range(NPAIR)]
            # v natural, ones-augmented per head ([64 dims, one]): [T, 8*65] bf16
            v_t = [poolB.tile([128, HL, 65], BF16, name=f"v_{i}") for i in range(NT)]
            for i in range(NT):
                nc.vector.memset(v_t[i][:, :, 64:65], 1.0)

            def v_chain(i):
                ps = ps_mm.tile([128, 512], F32, tag="mm")
                for cc in range(NC8):
                    nc.tensor.matmul(
                        ps[:],
                        xT[:, cc, 128 * i : 128 * i + 128],
                        wv_t[cc][:],
                        start=(cc == 0),
                        stop=(cc == NC8 - 1),
                    )
                with nc.allow_low_precision(reason="bf16 v evict"):
                    nc.vector.tensor_copy(
                        v_t[i][:, :, 0:64], ps[:].rearrange("p (h d) -> p h d", d=64)
                    )

            def kq_chain(p, t4):
                for wt, dst in ((wk_t, kT), (wq_t, qT)):
                    ps = ps_mm.tile([128, 512], F32, tag="mm")
                    for cc in range(NC8):
                        nc.tensor.matmul(
                            ps[:],
                            wt[cc][:, 128 * p : 128 * p + 128],
                            xT[:, cc, 512 * t4 : 512 * t4 + 512],
                            start=(cc == 0),
                            stop=(cc == NC8 - 1),
                        )
                    with nc.allow_low_precision(reason="bf16 kq evict"):
                        nc.scalar.activation(
                            dst[p][:, 512 * t4 : 512 * t4 + 512], ps[:], AF.Identity
                        )

            for i4 in range(NT // 4):
                x_t4 = work.tile([128, 4, C], BF16, tag="x_t", bufs=2)
                nc.sync.dma_start(
                    x_t4[:], bh(X_OFF + 512 * i4 * C,
                                "(k p c) -> p k c", k=4, p=128, c=C)
                )
                for k in range(4):
                    i = 4 * i4 + k
                    xh = ln_norm(x_t4[:, k, :])
                    ln_transpose(xT, xh, i)
                    v_chain(i)
                for p in range(NPAIR):
                    kq_chain(p, i4)

            if DEBUG_DUMP:
                nc.sync.dma_start(dbg_xT[:], xT[:])
                nc.sync.dma_start(dbg_kT[:], kT[0][:])
                nc.sync.dma_start(dbg_qT[:], qT[0][:])
                nc.sync.dma_start(dbg_v[:], v_t[0][:])

            # ---------------- phase C: attention ----------------
            ctxA.close()  # xT + wqkv free after v is built
            wpP = ctxW.enter_context(tc.tile_pool(name="wpP", bufs=1, side="right"))
            wp_q = [wpP.tile([128, 4, C], BF16, name=f"wp4_{g}") for g in range(2)]
            for g in range(2):
                nc.sync.dma_start(
                    wp_q[g][:],
                    bh(WP_OFF + 512 * g * C, "(k p m) -> p k m", k=4, p=128, m=C),
                )
            wp_t = [wp_q[i // 4][:, i % 4, :] for i in range(NC8)]

            poolC = ctxC.enter_context(tc.tile_pool(name="poolC", bufs=1, side="right"))
            # attnT [512, T] bf16; own T-half lives in cols [0:TH], peer in [TH:T]
            attnT = [poolC.tile([128, T], BF16, name=f"attnT_{p}") for p in range(NPAIR)]
            attnP = poolC.tile([128, NPAIR, TH], BF16, name="attnP")

            v_pid = nc.vector.partition_id()
            # local col offset for global t-chunk t4: (t4*512 + (pid%2)*1024) % 2048
            tc_off = [((v_pid % 2) * TH + (512 * t4)) % T for t4 in range(T // 512)]

            rs_pool = ctx.enter_context(tc.tile_pool(name="rs_dram", bufs=1, space="DRAM"))

            def rs_half(idx, lo):
                # ReduceScatter over pairs: exchange local peer cols [TH+lo : TH+lo+512]
                rs_in = rs_pool.tile([1024, 512], BF16, name=f"rs_in_{idx}")
                rs_out = rs_pool.tile([512, 512], BF16, name=f"rs_out_{idx}")
                rs_in4 = rs_in.rearrange("(s pp p) c -> pp p s c", s=2, p=128)
                for pp in range(NPAIR):
                    tmp = work.tile([128, 2, 512], BF16, tag="rs_tmp", bufs=1)
                    for sh in range(2):
                        with nc.allow_low_precision(reason="bf16 rs pack"):
                            nc.vector.tensor_scalar(
                                tmp[:, sh, :],
                                attnT[pp][:, TH + lo : TH + lo + 512],
                                selc[:, sh : sh + 1],
                                None,
                                ALU.mult,
                            )
                    nc.sync.dma_start(rs_in4[pp], tmp[:])
                if NO_COLLECTIVE:
                    nc.sync.dma_start(rs_out[:], rs_in[0:512, :])
                else:
                    nc.gpsimd.collective_compute(
                        "ReduceScatter",
                        ALU.add,
                        replica_groups=[[0, 1], [2, 3], [4, 5], [6, 7]],
                        ins=[rs_in[:]],
                        outs=[rs_out[:]],
                    )
                nc.gpsimd.dma_start(
                    attnP[:, :, lo : lo + 512],
                    rs_out.rearrange("(pp p) c -> p pp c", p=128),
                )

            # proj t2=0 staging: matmul+evict+transpose interleaved into the
            # attention tail; residual adds happen post-attention.
            pjT0 = [
                attn_sm.tile([128, 4, 128], BF16, name=f"pjT0_{cpt}", bufs=1)
                for cpt in range(NC8)
            ]

            def proj_mm(t2, cpt, dst, evict_dve=False):
                ps = ps_mm.tile([128, 512], F32, tag="mm")
                for cc in range(NC8):
                    rhs = (
                        attnT[cc][:, 512 * t2 : 512 * t2 + 512]
                        if cc < NPAIR
                        else attnP[:, cc - NPAIR, 512 * t2 : 512 * t2 + 512]
                    )
                    nc.tensor.matmul(
                        ps[:],
                        wp_t[cc][:, 128 * cpt : 128 * cpt + 128],
                        rhs,
                        start=(cc == 0),
                        stop=(cc == NC8 - 1),
                    )
                pj = work.tile([128, 512], BF16, tag="pj")
                with nc.allow_low_precision(reason="bf16 proj evict"):
                    if evict_dve:
                        nc.vector.tensor_copy(pj[:], ps[:])
                    else:
                        nc.scalar.activation(pj[:], ps[:], AF.Identity)
                nc.sync.dma_start(dst[:], pj[:], transpose=True)

            # softmax epilogue part B (PE rank-1 broadcast + normalize mults)
            # is software-pipelined one pair behind: it issues mid-way through
            # the NEXT pair's block loop so its reciprocals (DVE, ~0.5us) have
            # landed and the PE queue never head-of-line blocks on them.
            pend_epi = []

            def flush_epi():
                while pend_epi:
                    pend_epi.pop(0)()

            def make_epiB(p, t4, rA, rB, av_sb):
                def epiB():
                    rbA_ps = ps_mm.tile([128, 512], F32, tag="mm")
                    nc.tensor.matmul(
                        rbA_ps[:], ones_bf[0:1, :], rA[:], start=True, stop=True
                    )
                    rbB_ps = ps_mm.tile([128, 512], F32, tag="mm")
                    nc.tensor.matmul(
                        rbB_ps[:], ones_bf[0:1, :], rB[:], start=True, stop=True
                    )
                    rb_s = attn_sm.tile([128, 512], BF16, tag="rb_s")
                    with nc.allow_low_precision(reason="bf16 rb evict"):
                        nc.vector.tensor_copy(rb_s[0:64, :], rbA_ps[0:64, :])
                        nc.vector.tensor_copy(rb_s[64:128, :], rbB_ps[64:128, :])
                    with nc.allow_low_precision(reason="bf16 attn out"):
                        nc.vector.tensor_tensor(
                            attnT[p][:, bass.ds(tc_off[t4], 512)],
                            av_sb[:],
                            rb_s[:],
                            ALU.mult,
                        )
                return epiB

            for idx, t4 in enumerate(T4_ORDER):
                if idx == 2:
                    # local peer cols [TH : TH+512] (even: g2, odd: g0) complete
                    flush_epi()
                    rs_half(0, 0)
                for p in range(NPAIR):
                    hA, hB = 2 * p, 2 * p + 1
                    s_hi = 4 * (t4 + 1)
                    avA = ps_av.tile([65, 512], F32, tag="avA")
                    avB = ps_av.tile([65, 512], F32, tag="avB")
                    for sb in range(s_hi):
                        if sb == 1:
                            flush_epi()
                        # diagonal blocks only need t-columns >= 128j
                        j = sb - 4 * t4
                        lo = 128 * j if j > 0 else 0
                        psc = ps_sc.tile([128, 1024], F32, tag="sc")
                        nc.tensor.matmul(
                            psc[:, lo : 512],
                            kT[p][0:64, 128 * sb : 128 * sb + 128],
                            qT[p][0:64, 512 * t4 + lo : 512 * t4 + 512],
                            start=True,
                            stop=True,
                            tile_position=(0, 0),
                        )
                        nc.tensor.matmul(
                            psc[:, 512 + lo : 1024],
                            kT[p][64:128, 128 * sb : 128 * sb + 128],
                            qT[p][64:128, 512 * t4 + lo : 512 * t4 + 512],
                            start=True,
                            stop=True,
                            tile_position=(64, 0),
                        )
                        ee = exp_pool.tile([128, 1024], BF16, tag="ee")
                        psc3 = psc.rearrange("q (h t) -> q h t", t=512)
                        ee3 = ee.rearrange("q (h t) -> q h t", t=512)
                        with nc.allow_low_precision(reason="bf16 softmax weights"):
                            nc.scalar.activation(
                                ee3[:, :, lo:512], psc3[:, :, lo:512], AF.Exp
                            )
                        if j >= 0:
                            # causal mask: zero the above-diagonal weights of
                            # the [128, 128] triangle block
                            nc.vector.tensor_tensor(
                                ee3[:, :, lo : lo + 128],
                                ee3[:, :, lo : lo + 128],
                                tri[:],
                                ALU.mult,
                            )
                        nc.tensor.matmul(
                            avA[:, lo:512],
                            v_t[sb][:, hA, :],
                            ee[:, lo : 512],
                            start=(sb == 0),
                            stop=(sb == s_hi - 1),
                        )
                        nc.tensor.matmul(
                            avB[:, lo:512],
                            v_t[sb][:, hB, :],
                            ee[:, 512 + lo : 1024],
                            start=(sb == 0),
                            stop=(sb == s_hi - 1),
                        )
                    # epilogue part A: free the AV PSUM tiles + reciprocals
                    esA = attn_sm.tile([64, 512], BF16, tag="esA")
                    esB = attn_sm.tile([64, 512], BF16, tag="esB")
                    dAf = attn_sm.tile([1, 512], F32, tag="dAf", bufs=1)
                    dBf = attn_sm.tile([1, 512], F32, tag="dBf", bufs=1)
                    rAf = attn_sm.tile([1, 512], F32, tag="rAf", bufs=1)
                    rBf = attn_sm.tile([1, 512], F32, tag="rBf", bufs=1)
                    with nc.allow_low_precision(reason="softmax recip rounds"):
                        nc.vector.tensor_copy(esA[:], avA[0:64, :])
                        nc.vector.tensor_copy(dAf[:], avA[64:65, :])
                        nc.vector.tensor_copy(esB[:], avB[0:64, :])
                        nc.vector.tensor_copy(dBf[:], avB[64:65, :])
                    nc.vector.reciprocal_approx_fast(rAf[:], dAf[:])
                    nc.vector.reciprocal_approx_fast(rBf[:], dBf[:])
                    rA = attn_sm.tile([1, 512], BF16, tag="rA")
                    rB = attn_sm.tile([1, 512], BF16, tag="rB")
                    with nc.allow_low_precision(reason="bf16 recip"):
                        nc.vector.tensor_copy(rA[:], rAf[:])
                        nc.vector.tensor_copy(rB[:], rBf[:])
                    av_sb = attn_sm.tile([128, 512], BF16, tag="av_sb")
                    nc.sync.dma_start(av_sb[0:64, :], esA[:])
                    nc.sync.dma_start(av_sb[64:128, :], esB[:])
                    pend_epi.append(make_epiB(p, t4, rA, rB, av_sb))
                    # fill attention's PE bubbles with proj t2=0 chains,
                    # one per group so each PE detour stays under the exp
                    # stream's buffering depth (rs#1 lands ~26us after its
                    # trigger at idx==2 p==0, so the first fill waits a pair)
                    sched = {(2, 2): [0], (2, 3): [1],
                             (3, 0): [2], (3, 1): [3], (3, 2): [4, 5],
                             (3, 3): [6, 7]}
                    for cpt in sched.get((idx, p), []):
                        proj_mm(0, cpt, pjT0[cpt], evict_dve=True)
            flush_epi()
            # final RS half: local peer cols [TH+512 : T]
            rs_half(1, 512)

            # ---------------- phase E/F/G: proj + LN2 + FFN per t-half ----------------
            ctxB.close()  # kT/qT/v free after attention
            poolE = ctx.enter_context(tc.tile_pool(name="poolE", bufs=1, side="left"))
            x_own4 = [poolE.tile([128, 4, C], F32, name=f"xo4_{i}") for i in range(2)]
            for i in range(2):
                nc.sync.dma_start(
                    x_own4[i][:],
                    bf(XO_OFF + 512 * i * C, "(k p c) -> p k c", k=4, p=128, c=C),
                )
            x_own = [x_own4[i // 4][:, i % 4, :] for i in range(NTH)]
            x2 = x_own  # residual accumulated in place (bproj folded on host)

            poolF = ctx.enter_context(tc.tile_pool(name="poolF", bufs=1, side="left"))
            x2T = poolF.tile([128, NC8, TH], BF16, name="x2T")
            h1 = [poolF.tile([128, 512], BF16, name=f"h1_{m}") for m in range(NM)]

            def proj_add(t2, cpt, pjT):
                for tj in range(4):
                    tt = 4 * t2 + tj
                    nc.gpsimd.tensor_tensor(
                        x2[tt][:, 128 * cpt : 128 * cpt + 128],
                        pjT[:, tj, :],
                        x_own[tt][:, 128 * cpt : 128 * cpt + 128],
                        ALU.add,
                    )

            def proj_half(t2):
                for cpt in range(NC8):
                    pjT = work.tile([128, 4, 128], BF16, tag="pjT")
                    proj_mm(t2, cpt, pjT)
                    proj_add(t2, cpt, pjT)

            def ln2_half(t2):
                dst3 = x2T[:, :, 512 * t2 :]
                xh_p = None
                for i4 in range(4):
                    xh = ln_norm(x2[4 * t2 + i4][:])
                    if xh_p is not None:
                        ln_transpose(dst3, xh_p, i4 - 1)
                    xh_p = xh
                ln_transpose(dst3, xh_p, 3)

            if DEBUG_DUMP:
                for pp in range(NPAIR):
                    nc.sync.dma_start(dbg_attnT[pp], attnT[pp][:])
                nc.sync.dma_start(dbg_attnP[:], attnP[:])

            # ---------------- FFN ----------------
            # h1 of half 0 runs BEFORE proj_half(1): its ~110us of PE work
            # hides the second ReduceScatter's latency (proj t2=1 needs rs#2's
            # attnP). The W2 pool (32KB/partition) only fits after the
            # attention-side pools close, so h2 is a separate stage.
            def ffn_h1(t2, w1p, w1g0=None):
                for mg in range(NM // 4):  # 8 groups of 4 m-blocks
                    if mg == 0 and w1g0 is not None:
                        w1g = w1g0
                    else:
                        w1g = w1p.tile([128, NC8, 512], BF16, tag="w1g")
                        nc.sync.dma_start(
                            w1g[:],
                            bh(W1_OFF + mg * 128 * NC8 * 512,
                               "(p c m) -> p c m", p=128, c=NC8, m=512),
                        )
                    for mb in range(4):
                        m = 4 * mg + mb
                        ps = ps_mm.tile([128, 512], F32, tag="mm")
                        for cc in range(NC8):
                            nc.tensor.matmul(
                                ps[:],
                                w1g[:, cc, 128 * mb : 128 * mb + 128],
                                x2T[:, cc, 512 * t2 : 512 * t2 + 512],
                                start=(cc == 0),
                                stop=(cc == NC8 - 1),
                            )
                        with nc.allow_low_precision(reason="bf16 h1 evict"):
                            nc.vector.tensor_scalar(
                                h1[m][:], ps[:], b1c[:, m : m + 1], zero_c[:],
                                ALU.add, ALU.max,
                            )

            def ffn_h2(t2, w2p):
                for cp in range(2):
                    w2q = [
                        w2p.tile([128, 4, 512], BF16, tag="w2h", name=f"w2h_{t2}_{cp}_{q}")
                        for q in range(NM // 4)
                    ]
                    for q in range(NM // 4):
                        nc.sync.dma_start(
                            w2q[q][:],
                            bh(W2_OFF + (cp * (NM // 4) + q) * 128 * 4 * 512,
                               "(p j c) -> p j c", p=128, j=4, c=512),
                        )
                    for tt2 in range(4):
                        tt = 4 * t2 + tt2
                        ps = ps_mm.tile([128, 512], F32, tag="mm")
                        for m in range(NM):
                            nc.tensor.matmul(
                                ps[:],
                                h1[m][:, 128 * tt2 : 128 * tt2 + 128],
                                w2q[m // 4][:, m % 4, :],
                                start=(m == 0),
                                stop=False,
                            )
                        nc.tensor.matmul(
                            ps[:],
                            ones_bf[0:1, :],
                            b2r[0:1, 512 * cp : 512 * cp + 512],
                            start=False,
                            stop=True,
                        )
                        out_sb = work.tile([128, 512], F32, tag="out_sb")
                        nc.vector.tensor_tensor(
                            out_sb[:], ps[:], x2[tt][:, 512 * cp : 512 * cp + 512],
                            ALU.add,
                        )
                        nc.scalar.dma_start(
                            out[128 * tt : 128 * tt + 128, 512 * cp : 512 * cp + 512],
                            out_sb[:],
                        )

            with tc.tile_pool(name="w1pa", bufs=2, side="right") as w1pa:
                # W1 mg=0 prefetch rides the DMA engines while proj_add + LN2
                # (DVE/ACT) bridge toward the FFN
                w1g0 = w1pa.tile([128, NC8, 512], BF16, tag="w1g")
                nc.sync.dma_start(
                    w1g0[:], bh(W1_OFF, "(p c m) -> p c m", p=128, c=NC8, m=512)
                )
                for cpt in range(NC8):
                    proj_add(0, cpt, pjT0[cpt])
                ln2_half(0)
                ffn_h1(0, w1pa, w1g0=w1g0)
                proj_half(1)
            ctxC.close()
            ctxW.close()
            if DEBUG_DUMP:
                for i in range(NTH):
                    nc.sync.dma_start(dbg_x2[128 * i : 128 * i + 128, :], x2[i][:])
            with (
                tc.tile_pool(name="w1pb", bufs=2, side="right") as w1pb,
                tc.tile_pool(name="w2p", bufs=8, side="right") as w2p,
            ):
                # h2(0) only needs half-0 state, so it runs first and its PE
                # stream covers ln2_half(1)'s ACT/DVE latency
                ffn_h2(0, w2p)
                ln2_half(1)
                ffn_h1(1, w1pb)
                ffn_h2(1, w2p)

    nc.compile()
    return nc


def _get_program():
    global _PROGRAM
    if _PROGRAM is None:
        _PROGRAM = _build_program()
    return _PROGRAM


def make_in_maps(x, Wq, Wk, Wv, Wproj, bproj, ln1_g, ln1_b, ln2_g, ln2_b, W1, b1, W2, b2):
    """Host-side sharding: build the 8 per-core input maps."""
    x = np.asarray(x, np.float32)
    Wq = np.asarray(Wq, np.float32)
    Wk = np.asarray(Wk, np.float32)
    Wv = np.asarray(Wv, np.float32)
    Wproj = np.asarray(Wproj, np.float32)
    bproj = np.asarray(bproj, np.float32)
    ln1_g = np.asarray(ln1_g, np.float32)
    ln1_b = np.asarray(ln1_b, np.float32)
    ln2_g = np.asarray(ln2_g, np.float32)
    ln2_b = np.asarray(ln2_b, np.float32)
    W1 = np.asarray(W1, np.float32)
    b1 = np.asarray(b1, np.float32)
    W2 = np.asarray(W2, np.float32)
    b2 = np.asarray(b2, np.float32)

    assert np.all(ln1_b == 0.0) and np.all(ln2_b == 0.0), (
        "nonzero LN bias folding not implemented"
    )

    scale = 1.0 / np.sqrt(C)
    # [H, C, HS] -> g-folded, concat to [C, H*HS]
    Wq_f = (ln1_g[None, :, None] * Wq * scale).transpose(1, 0, 2).reshape(C, H * HS)
    Wk_f = (ln1_g[None, :, None] * Wk).transpose(1, 0, 2).reshape(C, H * HS)
    Wv_f = (ln1_g[None, :, None] * Wv).transpose(1, 0, 2).reshape(C, H * HS)
    W1_f = ln2_g[:, None] * W1

    # causal binary mask for the [128,128] diagonal triangle, both heads
    tri = np.zeros((128, 2, 128), np.float32)
    s_idx = np.arange(128)[:, None]
    t_idx = np.arange(128)[None, :]
    tri[:, 0, :] = (s_idx <= t_idx).astype(np.float32)
    tri[:, 1, :] = tri[:, 0, :]

    # W1 device-read order: per mg-group [p, c, m] with source row c*128+p,
    # col mg*512+m  ->  host layout [mg, p, c, m]
    w1_blob = (
        W1_f.astype(BF16_NP)
        .reshape(NC8, 128, NM // 4, 512)
        .transpose(2, 1, 0, 3)
        .ravel()
    )
    # W2 device-read order: per (cp, q) block [p, j, c2] with source row
    # 512q+128j+p, col 512cp+c2  ->  host layout [cp, q, p, j, c2]
    w2_blob = (
        W2.astype(BF16_NP)
        .reshape(NM // 4, 4, 128, 2, 512)
        .transpose(3, 0, 2, 1, 4)
        .ravel()
    )
    b1c_blob = b1.reshape(FF // 128, 128).T.astype(np.float32).ravel()

    in_maps = []
    for c in range(N_CORES):
        b = c // 2
        hg = c % 2
        cols = slice(hg * HL * HS, (hg + 1) * HL * HS)
        # Wproj rows permuted: own head block first, then peer's
        own = Wproj[hg * HL * HS : (hg + 1) * HL * HS, :]
        peer = Wproj[(1 - hg) * HL * HS : (2 - hg) * HL * HS, :]
        selc = np.zeros((128, 2), np.float32)
        selc[:, 0] = hg
        selc[:, 1] = 1 - hg

        blob_h = np.zeros(BF_TOTAL, BF16_NP)
        blob_h[X_OFF : X_OFF + T * C] = x[b].astype(BF16_NP).ravel()
        blob_h[WQ_OFF : WQ_OFF + C * HL * HS] = Wq_f[:, cols].astype(BF16_NP).ravel()
        blob_h[WK_OFF : WK_OFF + C * HL * HS] = Wk_f[:, cols].astype(BF16_NP).ravel()
        blob_h[WV_OFF : WV_OFF + C * HL * HS] = Wv_f[:, cols].astype(BF16_NP).ravel()
        blob_h[WP_OFF : WP_OFF + C * C] = (
            np.concatenate([own, peer], axis=0).astype(BF16_NP).ravel()
        )
        blob_h[W1_OFF : W1_OFF + C * FF] = w1_blob
        blob_h[W2_OFF : W2_OFF + FF * C] = w2_blob
        blob_h[TRI_OFF : TRI_OFF + 128 * 2 * 128] = tri.astype(BF16_NP).ravel()
        blob_h[B2R_OFF : B2R_OFF + C] = b2.astype(BF16_NP).ravel()
        blob_h[ONES_OFF : ONES_OFF + 128] = np.ones(128, BF16_NP)

        blob_f = np.zeros(F32_TOTAL, np.float32)
        blob_f[XO_OFF : XO_OFF + TH * C] = (
            x[b, hg * TH : (hg + 1) * TH, :] + bproj[None, :]
        ).astype(np.float32).ravel()
        blob_f[B1C_OFF : B1C_OFF + 128 * NM] = b1c_blob
        blob_f[SELC_OFF : SELC_OFF + 256] = selc.ravel()

        in_maps.append({"blob_h": blob_h, "blob_f": blob_f})
    return in_maps


def assemble(results):
    out = np.empty((B, T, C), np.float32)
    for c in range(N_CORES):
        b, hg = c // 2, c % 2
        out[b, hg * TH : (hg + 1) * TH, :] = results[c]["out_half"]
    return out


def kernel(**inputs):
    from concourse import bass2jax

    nc = _get_program()
    in_maps = make_in_maps(**inputs)
    results = bass2jax.run_bass_via_pjrt(nc, in_maps, n_cores=N_CORES)
    return assemble(results)



# revision 37
# speedup vs baseline: 1.0479x; 1.0479x over previous
"""Trainium2 Bass kernel for a dense transformer block (B=4, T=2048, C=1024, H=16).

Sharding (8 cores): core c handles batch b=c//2 and head-group hg=c%2
(8 heads). Each core computes LN1 + QKV + causal attention for its 8 heads
over the full T=2048, then a 2-core ReduceScatter exchanges attnT halves
within each (batch) pair so core c finishes proj + LN2 + FFN for its own
T-half (rows hg*1024 .. hg*1024+1024) with the full set of 16 heads.

Structure (v3):
- All inputs ship as TWO dram blobs (bf16 + f32): per-call dispatch cost
  through the axon tunnel scales with operand count (~36us/operand), and the
  host lays each tensor out in exactly the order the device DMAs it, so every
  transfer is contiguous per partition.
- LN1 -> transpose -> V -> KQ are interleaved per 4-tile group so PE work
  starts after one tile's LN chain instead of after all sixteen. LN uses
  reciprocal_approx_fast (SBUF input only! its bitwise NR seed reads garbage
  from PSUM's e10m23 accumulators).
- Attention: scores in PSUM f32 (diagonal blocks subranged), exp on ACT ->
  ee bf16, causal mask post-exp as a {0,1} multiply on the triangle block,
  AV matmuls bf16 with ones-augmented V so the denominator falls out in row
  64. Epilogue is split: part A (es eviction + denominator copy to SBUF +
  approx reciprocal) frees the AV PSUM immediately; part B (PE rank-1
  broadcast + normalize multiply) is deferred one pair so the PE queue never
  head-of-line blocks on the reciprocal.
- t4 order (2,0,3,1) fires ReduceScatter #1 mid-attention; proj t2=0 chains
  fill attention's PE bubbles (scheduled a pair after the RS trigger to
  cover its ~26us latency).
- Tail: FFN h1(half0) runs before proj_half(1), hiding RS#2; h2(half0) runs
  before ln2_half(1) so LN2 hides under PE; W1 mg=0 is prefetched. All
  dma-transpose issues ride the sync queue (~1.3us issue cost each would
  otherwise clog ACT).
- FFN stays bf16: fp8e4m3 DoubleRow was tried and REVERTED — quantization
  noise does not average out in random-sign reductions (signal and noise
  both grow as sqrt(K)), so fp8xfp8 costs ~5% rms per layer, ~3e-2 on the
  output vs the 2e-2 gate.
"""

import sys
import numpy as np

for _p in ("/opt/trn_rl_repo",):
    if _p not in sys.path:
        sys.path.append(_p)

import concourse.bass as bass
import concourse.bacc as bacc
import concourse.tile as tile
import concourse.mybir as mybir

dt = mybir.dt
AF = mybir.ActivationFunctionType
ALU = mybir.AluOpType
F32 = dt.float32
F32R = dt.float32r
BF16 = dt.bfloat16

N_CORES = 8
B, T, C = 4, 2048, 1024
H, HS = 16, 64
HL = 8            # heads per core (local)
TH = T // 2       # t-half (rows per core for proj/FFN)
FF = 4 * C        # 4096
EPS = 1e-5

BF16_NP = dt.np(BF16)

_PROGRAM = None
NO_COLLECTIVE = False  # replace RS with local DMA (for TimelineSim)
DEBUG_DUMP = False

NT = T // 128          # 16 t-tiles (full T)
NTH = TH // 128        # 8 t-tiles (own half)
NC8 = C // 128         # 8 c-chunks
NPAIR = HL // 2        # 4 head pairs
NM = FF // 128         # 32 FFN m-blocks
T4_ORDER = (2, 0, 3, 1)

# ---- packed-input blob offsets (elements) ----
# All inputs ship as TWO dram tensors (one bf16, one f32): per-call dispatch
# cost through the axon tunnel scales with operand count (~36us/operand), so
# 13 separate tensors -> 2 blobs. Layout on host matches the exact order the
# device DMAs each block, so every transfer is contiguous per partition.
X_OFF = 0
WQ_OFF = X_OFF + T * C                      # 2_097_152
WK_OFF = WQ_OFF + C * HL * HS               # +524_288
WV_OFF = WK_OFF + C * HL * HS
WP_OFF = WV_OFF + C * HL * HS
W1_OFF = WP_OFF + C * C
W2_OFF = W1_OFF + C * FF
TRI_OFF = W2_OFF + FF * C
B2R_OFF = TRI_OFF + 128 * 2 * 128
ONES_OFF = B2R_OFF + C
BF_TOTAL = ONES_OFF + 128

XO_OFF = 0
B1C_OFF = XO_OFF + TH * C
SELC_OFF = B1C_OFF + 128 * NM
F32_TOTAL = SELC_OFF + 128 * 2


def _build_program():
    nc = bacc.Bacc(
        "TRN2",
        target_bir_lowering=False,
        debug=False,
        num_devices=N_CORES,
        enable_partition_id=True,
    )

    # ---- I/O (packed: see blob offset table above) ----
    blob_h = nc.dram_tensor("blob_h", [BF_TOTAL], BF16, kind="ExternalInput")
    blob_f = nc.dram_tensor("blob_f", [F32_TOTAL], F32, kind="ExternalInput")
    out = nc.dram_tensor("out_half", [TH, C], F32, kind="ExternalOutput")

    def bh(off, pattern, **axes):
        n = 1
        for v in axes.values():
            n *= v
        return blob_h[off : off + n].rearrange(pattern, **axes)

    def bf(off, pattern, **axes):
        n = 1
        for v in axes.values():
            n *= v
        return blob_f[off : off + n].rearrange(pattern, **axes)
    if DEBUG_DUMP:
        dbg_xT = nc.dram_tensor("dbg_xT", [128, NC8, T], BF16, kind="ExternalOutput")
        dbg_kT = nc.dram_tensor("dbg_kT", [128, T], BF16, kind="ExternalOutput")
        dbg_qT = nc.dram_tensor("dbg_qT", [128, T], BF16, kind="ExternalOutput")
        dbg_v = nc.dram_tensor("dbg_v", [128, HL, 65], BF16, kind="ExternalOutput")
        dbg_attnT = nc.dram_tensor("dbg_attnT", [NPAIR, 128, T], BF16, kind="ExternalOutput")
        dbg_attnP = nc.dram_tensor("dbg_attnP", [128, NPAIR, TH], BF16, kind="ExternalOutput")
        dbg_x2 = nc.dram_tensor("dbg_x2", [TH, C], F32, kind="ExternalOutput")

    with tile.TileContext(nc) as tc:
        from contextlib import ExitStack

        ctx = ExitStack()
        with ctx:
            # ---------------- pools ----------------
            consts = ctx.enter_context(tc.tile_pool(name="consts", bufs=1))
            ps_mm = ctx.enter_context(tc.tile_pool(name="ps_mm", bufs=2, space="PSUM"))
            ps_sc = ctx.enter_context(tc.tile_pool(name="ps_sc", bufs=2, space="PSUM"))
            ps_av = ctx.enter_context(tc.tile_pool(name="ps_av", bufs=1, space="PSUM"))
            dram = ctx.enter_context(tc.tile_pool(name="dram", bufs=1, space="DRAM"))

            work = ctx.enter_context(tc.tile_pool(name="work", bufs=2, side="left"))
            ln_pool = ctx.enter_context(tc.tile_pool(name="ln", bufs=6, side="left"))
            exp_pool = ctx.enter_context(tc.tile_pool(name="exp", bufs=3, side="left"))
            attn_sm = ctx.enter_context(
                tc.tile_pool(name="attn_sm", bufs=2, side="left")
            )
            # phase-scoped pools (strict LIFO per side):
            # right: poolA -> (close) -> wpP -> poolC -> (close both) -> w1p/w2p
            # left:  poolB -> (close) -> poolE -> poolF
            ctxA = ExitStack()   # xT + wq/wk/wv         — dies after v production
            ctxB = ExitStack()   # kT, qT, v             — dies after attention
            ctxW = ExitStack()   # wproj                 — dies after proj
            ctxC = ExitStack()   # attnT, attnP          — dies after proj
            poolA = ctxA.enter_context(tc.tile_pool(name="poolA", bufs=1, side="right"))
            poolB = ctxB.enter_context(tc.tile_pool(name="poolB", bufs=1, side="left"))

            # ---------------- constants ----------------
            tri = consts.tile([128, 2, 128], BF16, name="tri_t")
            nc.sync.dma_start(tri[:], bh(TRI_OFF, "(p k m) -> p k m", p=128, k=2, m=128))
            b1c = consts.tile([128, NM], F32, name="b1c_t")
            nc.sync.dma_start(b1c[:], bf(B1C_OFF, "(p m) -> p m", p=128, m=NM))
            b2r = consts.tile([1, C], BF16, name="b2r_t")
            nc.sync.dma_start(b2r[:], bh(B2R_OFF, "(a c) -> a c", a=1, c=C))
            ones_bf = consts.tile([1, 128], BF16, name="ones_bf_t")
            nc.sync.dma_start(ones_bf[:], bh(ONES_OFF, "(a c) -> a c", a=1, c=128))
            selc = consts.tile([128, 2], F32, name="selc_t")
            nc.sync.dma_start(selc[:], bf(SELC_OFF, "(p s) -> p s", p=128, s=2))
            eps_t = consts.tile([128, 1], F32, name="eps_t")
            nc.vector.memset(eps_t[:], EPS)
            neg1 = consts.tile([128, 1], F32, name="neg1_t")
            nc.vector.memset(neg1[:], -1.0)
            zero_c = consts.tile([128, 1], F32, name="zero_c")
            nc.vector.memset(zero_c[:], 0.0)

            # qkv weights, feature-major: [128, 4, 512] quads (one DMA each)
            wq_q = [poolA.tile([128, 4, HL * HS], BF16, name=f"wq4_{g}") for g in range(2)]
            wk_q = [poolA.tile([128, 4, HL * HS], BF16, name=f"wk4_{g}") for g in range(2)]
            wv_q = [poolA.tile([128, 4, HL * HS], BF16, name=f"wv4_{g}") for g in range(2)]
            for g in range(2):
                for wt_q, w_off in ((wq_q, WQ_OFF), (wk_q, WK_OFF), (wv_q, WV_OFF)):
                    nc.sync.dma_start(
                        wt_q[g][:],
                        bh(w_off + 512 * g * (HL * HS),
                           "(k p m) -> p k m", k=4, p=128, m=HL * HS),
                    )
            wq_t = [wq_q[i // 4][:, i % 4, :] for i in range(NC8)]
            wk_t = [wk_q[i // 4][:, i % 4, :] for i in range(NC8)]
            wv_t = [wv_q[i // 4][:, i % 4, :] for i in range(NC8)]

            def ln_norm(x_sb):
                """LayerNorm x_sb [128, C] f32 -> xh bf16 (stats on DVE,
                normalize on ACT)."""
                st = ln_pool.tile([128, 2, 6], F32, tag="st")
                nc.vector.bn_stats(st[:, 0, :], x_sb[:, 0:512])
                nc.vector.bn_stats(st[:, 1, :], x_sb[:, 512:1024])
                mv = ln_pool.tile([128, 2], F32, tag="mv")
                nc.vector.bn_aggr(mv[:], st[:])
                sd = ln_pool.tile([128, 1], F32, tag="sd")
                nc.scalar.activation(sd[:], mv[:, 1:2], AF.Sqrt, bias=eps_t[:])
                rs = ln_pool.tile([128, 1], F32, tag="rs")
                nc.vector.reciprocal_approx_fast(rs[:], sd[:])
                bb = ln_pool.tile([128, 1], F32, tag="bb")
                nc.vector.tensor_scalar(
                    bb[:], mv[:, 0:1], rs[:], neg1[:], ALU.mult, ALU.mult
                )
                xh = work.tile([128, C], BF16, tag="xh", bufs=3)
                with nc.allow_low_precision(reason="bf16 ln out"):
                    nc.scalar.activation(
                        xh[:], x_sb[:], AF.Identity, bias=bb[:], scale=rs[:]
                    )
                return xh

            def ln_transpose(dst3, xh, i):
                # xbar transpose: out[p, j, t] = xh[t, 128j + p]
                # issued on the sync queue: descriptor-gen costs ~1.3us of the
                # issuing engine, and ACT is the busy one
                nc.sync.dma_start(
                    dst3[:, :, 128 * i : 128 * i + 128], xh[:], transpose=True
                )

            # ---------------- phase A+B: LN1 -> xT, V + KQ interleaved ------
            # V for tile i needs only tile i's transpose, and KQ for t4-chunk
            # g needs tiles 4g..4g+3 — so PE work starts after ONE tile's LN
            # chain instead of after all sixteen.
            xT = poolA.tile([128, NC8, T], BF16, name="xT")  # x-hat transposed
            kT = [poolB.tile([128, T], BF16, name=f"kT_{p}") for p in range(NPAIR)]
            qT = [poolB.tile([128, T], BF16, name=f"qT_{p}") for p in range(NPAIR)]
            # v natural, ones-augmented per head ([64 dims, one]): [T, 8*65] bf16
            v_t = [poolB.tile([128, HL, 65], BF16, name=f"v_{i}") for i in range(NT)]
            for i in range(NT):
                nc.vector.memset(v_t[i][:, :, 64:65], 1.0)

            def v_chain(i):
                ps = ps_mm.tile([128, 512], F32, tag="mm")
                for cc in range(NC8):
                    nc.tensor.matmul(
                        ps[:],
                        xT[:, cc, 128 * i : 128 * i + 128],
                        wv_t[cc][:],
                        start=(cc == 0),
                        stop=(cc == NC8 - 1),
                    )
                with nc.allow_low_precision(reason="bf16 v evict"):
                    nc.vector.tensor_copy(
                        v_t[i][:, :, 0:64], ps[:].rearrange("p (h d) -> p h d", d=64)
                    )

            def kq_chain(p, t4):
                for wt, dst in ((wk_t, kT), (wq_t, qT)):
                    ps = ps_mm.tile([128, 512], F32, tag="mm")
                    for cc in range(NC8):
                        nc.tensor.matmul(
                            ps[:],
                            wt[cc][:, 128 * p : 128 * p + 128],
                            xT[:, cc, 512 * t4 : 512 * t4 + 512],
                            start=(cc == 0),
                            stop=(cc == NC8 - 1),
                        )
                    with nc.allow_low_precision(reason="bf16 kq evict"):
                        nc.scalar.activation(
                            dst[p][:, 512 * t4 : 512 * t4 + 512], ps[:], AF.Identity
                        )

            for i4 in range(NT // 4):
                x_t4 = work.tile([128, 4, C], BF16, tag="x_t", bufs=2)
                nc.sync.dma_start(
                    x_t4[:], bh(X_OFF + 512 * i4 * C,
                                "(k p c) -> p k c", k=4, p=128, c=C)
                )
                for k in range(4):
                    i = 4 * i4 + k
                    xh = ln_norm(x_t4[:, k, :])
                    ln_transpose(xT, xh, i)
                    v_chain(i)
                for p in range(NPAIR):
                    kq_chain(p, i4)

            if DEBUG_DUMP:
                nc.sync.dma_start(dbg_xT[:], xT[:])
                nc.sync.dma_start(dbg_kT[:], kT[0][:])
                nc.sync.dma_start(dbg_qT[:], qT[0][:])
                nc.sync.dma_start(dbg_v[:], v_t[0][:])

            # ---------------- phase C: attention ----------------
            ctxA.close()  # xT + wqkv free after v is built
            wpP = ctxW.enter_context(tc.tile_pool(name="wpP", bufs=1, side="right"))
            wp_q = [wpP.tile([128, 4, C], BF16, name=f"wp4_{g}") for g in range(2)]
            for g in range(2):
                nc.sync.dma_start(
                    wp_q[g][:],
                    bh(WP_OFF + 512 * g * C, "(k p m) -> p k m", k=4, p=128, m=C),
                )
            wp_t = [wp_q[i // 4][:, i % 4, :] for i in range(NC8)]

            poolC = ctxC.enter_context(tc.tile_pool(name="poolC", bufs=1, side="right"))
            # attnT [512, T] bf16; own T-half lives in cols [0:TH], peer in [TH:T]
            attnT = [poolC.tile([128, T], BF16, name=f"attnT_{p}") for p in range(NPAIR)]
            attnP = poolC.tile([128, NPAIR, TH], BF16, name="attnP")

            v_pid = nc.vector.partition_id()
            # local col offset for global t-chunk t4: (t4*512 + (pid%2)*1024) % 2048
            tc_off = [((v_pid % 2) * TH + (512 * t4)) % T for t4 in range(T // 512)]

            rs_pool = ctx.enter_context(tc.tile_pool(name="rs_dram", bufs=1, space="DRAM"))

            def rs_half(idx, lo):
                # ReduceScatter over pairs: exchange local peer cols [TH+lo : TH+lo+512]
                rs_in = rs_pool.tile([1024, 512], BF16, name=f"rs_in_{idx}")
                rs_out = rs_pool.tile([512, 512], BF16, name=f"rs_out_{idx}")
                rs_in4 = rs_in.rearrange("(s pp p) c -> pp p s c", s=2, p=128)
                for pp in range(NPAIR):
                    tmp = work.tile([128, 2, 512], BF16, tag="rs_tmp", bufs=1)
                    for sh in range(2):
                        with nc.allow_low_precision(reason="bf16 rs pack"):
                            nc.vector.tensor_scalar(
                                tmp[:, sh, :],
                                attnT[pp][:, TH + lo : TH + lo + 512],
                                selc[:, sh : sh + 1],
                                None,
                                ALU.mult,
                            )
                    nc.sync.dma_start(rs_in4[pp], tmp[:])
                if NO_COLLECTIVE:
                    nc.sync.dma_start(rs_out[:], rs_in[0:512, :])
                else:
                    nc.gpsimd.collective_compute(
                        "ReduceScatter",
                        ALU.add,
                        replica_groups=[[0, 1], [2, 3], [4, 5], [6, 7]],
                        ins=[rs_in[:]],
                        outs=[rs_out[:]],
                    )
                nc.gpsimd.dma_start(
                    attnP[:, :, lo : lo + 512],
                    rs_out.rearrange("(pp p) c -> p pp c", p=128),
                )

            # proj t2=0 staging: matmul+evict+transpose interleaved into the
            # attention tail; residual adds happen post-attention.
            pjT0 = [
                attn_sm.tile([128, 4, 128], BF16, name=f"pjT0_{cpt}", bufs=1)
                for cpt in range(NC8)
            ]

            def proj_mm(t2, cpt, dst, evict_dve=False):
                ps = ps_mm.tile([128, 512], F32, tag="mm")
                for cc in range(NC8):
                    rhs = (
                        attnT[cc][:, 512 * t2 : 512 * t2 + 512]
                        if cc < NPAIR
                        else attnP[:, cc - NPAIR, 512 * t2 : 512 * t2 + 512]
                    )
                    nc.tensor.matmul(
                        ps[:],
                        wp_t[cc][:, 128 * cpt : 128 * cpt + 128],
                        rhs,
                        start=(cc == 0),
                        stop=(cc == NC8 - 1),
                    )
                pj = work.tile([128, 512], BF16, tag="pj")
                with nc.allow_low_precision(reason="bf16 proj evict"):
                    if evict_dve:
                        nc.vector.tensor_copy(pj[:], ps[:])
                    else:
                        nc.scalar.activation(pj[:], ps[:], AF.Identity)
                nc.sync.dma_start(dst[:], pj[:], transpose=True)

            # softmax epilogue part B (PE rank-1 broadcast + normalize mults)
            # is software-pipelined one pair behind: it issues mid-way through
            # the NEXT pair's block loop so its reciprocals (DVE, ~0.5us) have
            # landed and the PE queue never head-of-line blocks on them.
            pend_epi = []

            def flush_epi():
                while pend_epi:
                    pend_epi.pop(0)()

            def make_epiB(p, t4, rA, rB, av_sb):
                def epiB():
                    rbA_ps = ps_mm.tile([128, 512], F32, tag="mm")
                    nc.tensor.matmul(
                        rbA_ps[:], ones_bf[0:1, :], rA[:], start=True, stop=True
                    )
                    rbB_ps = ps_mm.tile([128, 512], F32, tag="mm")
                    nc.tensor.matmul(
                        rbB_ps[:], ones_bf[0:1, :], rB[:], start=True, stop=True
                    )
                    rb_s = attn_sm.tile([128, 512], BF16, tag="rb_s")
                    with nc.allow_low_precision(reason="bf16 rb evict"):
                        nc.vector.tensor_copy(rb_s[0:64, :], rbA_ps[0:64, :])
                        nc.vector.tensor_copy(rb_s[64:128, :], rbB_ps[64:128, :])
                    with nc.allow_low_precision(reason="bf16 attn out"):
                        nc.vector.tensor_tensor(
                            attnT[p][:, bass.ds(tc_off[t4], 512)],
                            av_sb[:],
                            rb_s[:],
                            ALU.mult,
                        )
                return epiB

            for idx, t4 in enumerate(T4_ORDER):
                if idx == 2:
                    # local peer cols [TH : TH+512] (even: g2, odd: g0) complete
                    flush_epi()
                    rs_half(0, 0)
                for p in range(NPAIR):
                    hA, hB = 2 * p, 2 * p + 1
                    s_hi = 4 * (t4 + 1)
                    avA = ps_av.tile([65, 512], F32, tag="avA")
                    avB = ps_av.tile([65, 512], F32, tag="avB")
                    for sb in range(s_hi):
                        if sb == 1:
                            flush_epi()
                        # diagonal blocks only need t-columns >= 128j
                        j = sb - 4 * t4
                        lo = 128 * j if j > 0 else 0
                        psc = ps_sc.tile([128, 1024], F32, tag="sc")
                        nc.tensor.matmul(
                            psc[:, lo : 512],
                            kT[p][0:64, 128 * sb : 128 * sb + 128],
                            qT[p][0:64, 512 * t4 + lo : 512 * t4 + 512],
                            start=True,
                            stop=True,
                            tile_position=(0, 0),
                        )
                        nc.tensor.matmul(
                            psc[:, 512 + lo : 1024],
                            kT[p][64:128, 128 * sb : 128 * sb + 128],
                            qT[p][64:128, 512 * t4 + lo : 512 * t4 + 512],
                            start=True,
                            stop=True,
                            tile_position=(64, 0),
                        )
                        ee = exp_pool.tile([128, 1024], BF16, tag="ee")
                        psc3 = psc.rearrange("q (h t) -> q h t", t=512)
                        ee3 = ee.rearrange("q (h t) -> q h t", t=512)
                        with nc.allow_low_precision(reason="bf16 softmax weights"):
                            nc.scalar.activation(
                                ee3[:, :, lo:512], psc3[:, :, lo:512], AF.Exp
                            )
                        if j >= 0:
                            # causal mask: zero the above-diagonal weights of
                            # the [128, 128] triangle block
                            nc.vector.tensor_tensor(
                                ee3[:, :, lo : lo + 128],
                                ee3[:, :, lo : lo + 128],
                                tri[:],
                                ALU.mult,
                            )
                        nc.tensor.matmul(
                            avA[:, lo:512],
                            v_t[sb][:, hA, :],
                            ee[:, lo : 512],
                            start=(sb == 0),
                            stop=(sb == s_hi - 1),
                        )
                        nc.tensor.matmul(
                            avB[:, lo:512],
                            v_t[sb][:, hB, :],
                            ee[:, 512 + lo : 1024],
                            start=(sb == 0),
                            stop=(sb == s_hi - 1),
                        )
                    # epilogue part A: free the AV PSUM tiles + reciprocals
                    esA = attn_sm.tile([64, 512], BF16, tag="esA")
                    esB = attn_sm.tile([64, 512], BF16, tag="esB")
                    dAf = attn_sm.tile([1, 512], F32, tag="dAf", bufs=1)
                    dBf = attn_sm.tile([1, 512], F32, tag="dBf", bufs=1)
                    rAf = attn_sm.tile([1, 512], F32, tag="rAf", bufs=1)
                    rBf = attn_sm.tile([1, 512], F32, tag="rBf", bufs=1)
                    with nc.allow_low_precision(reason="softmax recip rounds"):
                        nc.vector.tensor_copy(esA[:], avA[0:64, :])
                        nc.vector.tensor_copy(dAf[:], avA[64:65, :])
                        nc.vector.tensor_copy(esB[:], avB[0:64, :])
                        nc.vector.tensor_copy(dBf[:], avB[64:65, :])
                    nc.vector.reciprocal_approx_fast(rAf[:], dAf[:])
                    nc.vector.reciprocal_approx_fast(rBf[:], dBf[:])
                    rA = attn_sm.tile([1, 512], BF16, tag="rA")
                    rB = attn_sm.tile([1, 512], BF16, tag="rB")
                    with nc.allow_low_precision(reason="bf16 recip"):
                        nc.vector.tensor_copy(rA[:], rAf[:])
                        nc.vector.tensor_copy(rB[:], rBf[:])
                    av_sb = attn_sm.tile([128, 512], BF16, tag="av_sb")
                    nc.sync.dma_start(av_sb[0:64, :], esA[:])
                    nc.sync.dma_start(av_sb[64:128, :], esB[:])
                    pend_epi.append(make_epiB(p, t4, rA, rB, av_sb))
                    # fill attention's PE bubbles with proj t2=0 chains,
                    # one per group so each PE detour stays under the exp
                    # stream's buffering depth (rs#1 lands ~26us after its
                    # trigger at idx==2 p==0, so the first fill waits a pair)
                    sched = {(2, 2): [0], (2, 3): [1],
                             (3, 0): [2], (3, 1): [3], (3, 2): [4, 5],
                             (3, 3): [6, 7]}
                    for cpt in sched.get((idx, p), []):
                        proj_mm(0, cpt, pjT0[cpt], evict_dve=True)
            flush_epi()
            # final RS half: local peer cols [TH+512 : T]
            rs_half(1, 512)

            # ---------------- phase E/F/G: proj + LN2 + FFN per t-half ----------------
            ctxB.close()  # kT/qT/v free after attention
            poolE = ctx.enter_context(tc.tile_pool(name="poolE", bufs=1, side="left"))
            x_own4 = [poolE.tile([128, 4, C], F32, name=f"xo4_{i}") for i in range(2)]
            for i in range(2):
                nc.sync.dma_start(
                    x_own4[i][:],
                    bf(XO_OFF + 512 * i * C, "(k p c) -> p k c", k=4, p=128, c=C),
                )
            x_own = [x_own4[i // 4][:, i % 4, :] for i in range(NTH)]
            x2 = x_own  # residual accumulated in place (bproj folded on host)

            poolF = ctx.enter_context(tc.tile_pool(name="poolF", bufs=1, side="left"))
            x2T = poolF.tile([128, NC8, TH], BF16, name="x2T")
            h1 = [poolF.tile([128, 512], BF16, name=f"h1_{m}") for m in range(NM)]

            def proj_add(t2, cpt, pjT):
                for tj in range(4):
                    tt = 4 * t2 + tj
                    nc.gpsimd.tensor_tensor(
                        x2[tt][:, 128 * cpt : 128 * cpt + 128],
                        pjT[:, tj, :],
                        x_own[tt][:, 128 * cpt : 128 * cpt + 128],
                        ALU.add,
                    )

            def proj_half(t2):
                for cpt in range(NC8):
                    pjT = work.tile([128, 4, 128], BF16, tag="pjT")
                    proj_mm(t2, cpt, pjT)
                    proj_add(t2, cpt, pjT)

            def ln2_half(t2):
                dst3 = x2T[:, :, 512 * t2 :]
                xh_p = None
                for i4 in range(4):
                    xh = ln_norm(x2[4 * t2 + i4][:])
                    if xh_p is not None:
                        ln_transpose(dst3, xh_p, i4 - 1)
                    xh_p = xh
                ln_transpose(dst3, xh_p, 3)

            if DEBUG_DUMP:
                for pp in range(NPAIR):
                    nc.sync.dma_start(dbg_attnT[pp], attnT[pp][:])
                nc.sync.dma_start(dbg_attnP[:], attnP[:])

            # ---------------- FFN ----------------
            # h1 of half 0 runs BEFORE proj_half(1): its ~110us of PE work
            # hides the second ReduceScatter's latency (proj t2=1 needs rs#2's
            # attnP). The W2 pool (32KB/partition) only fits after the
            # attention-side pools close, so h2 is a separate stage.
            def ffn_h1(t2, w1p, w1g0=None):
                for mg in range(NM // 4):  # 8 groups of 4 m-blocks
                    if mg == 0 and w1g0 is not None:
                        w1g = w1g0
                    else:
                        w1g = w1p.tile([128, NC8, 512], BF16, tag="w1g")
                        nc.sync.dma_start(
                            w1g[:],
                            bh(W1_OFF + mg * 128 * NC8 * 512,
                               "(p c m) -> p c m", p=128, c=NC8, m=512),
                        )
                    for mb in range(4):
                        m = 4 * mg + mb
                        ps = ps_mm.tile([128, 512], F32, tag="mm")
                        for cc in range(NC8):
                            nc.tensor.matmul(
                                ps[:],
                                w1g[:, cc, 128 * mb : 128 * mb + 128],
                                x2T[:, cc, 512 * t2 : 512 * t2 + 512],
                                start=(cc == 0),
                                stop=(cc == NC8 - 1),
                            )
                        with nc.allow_low_precision(reason="bf16 h1 evict"):
                            nc.vector.tensor_scalar(
                                h1[m][:], ps[:], b1c[:, m : m + 1], zero_c[:],
                                ALU.add, ALU.max,
                            )

            def ffn_h2(t2, w2p):
                for cp in range(2):
                    w2q = [
                        w2p.tile([128, 4, 512], BF16, tag="w2h", name=f"w2h_{t2}_{cp}_{q}")
                        for q in range(NM // 4)
                    ]
                    for q in range(NM // 4):
                        nc.sync.dma_start(
                            w2q[q][:],
                            bh(W2_OFF + (cp * (NM // 4) + q) * 128 * 4 * 512,
                               "(p j c) -> p j c", p=128, j=4, c=512),
                        )
                    for tt2 in range(4):
                        tt = 4 * t2 + tt2
                        ps = ps_mm.tile([128, 512], F32, tag="mm")
                        for m in range(NM):
                            nc.tensor.matmul(
                                ps[:],
                                h1[m][:, 128 * tt2 : 128 * tt2 + 128],
                                w2q[m // 4][:, m % 4, :],
                                start=(m == 0),
                                stop=False,
                            )
                        nc.tensor.matmul(
                            ps[:],
                            ones_bf[0:1, :],
                            b2r[0:1, 512 * cp : 512 * cp + 512],
                            start=False,
                            stop=True,
                        )
                        out_sb = work.tile([128, 512], F32, tag="out_sb")
                        nc.vector.tensor_tensor(
                            out_sb[:], ps[:], x2[tt][:, 512 * cp : 512 * cp + 512],
                            ALU.add,
                        )
                        nc.scalar.dma_start(
                            out[128 * tt : 128 * tt + 128, 512 * cp : 512 * cp + 512],
                            out_sb[:],
                        )

            with tc.tile_pool(name="w1pa", bufs=2, side="right") as w1pa:
                # W1 mg=0 prefetch rides the DMA engines while proj_add + LN2
                # (DVE/ACT) bridge toward the FFN
                w1g0 = w1pa.tile([128, NC8, 512], BF16, tag="w1g")
                nc.sync.dma_start(
                    w1g0[:], bh(W1_OFF, "(p c m) -> p c m", p=128, c=NC8, m=512)
                )
                for cpt in range(NC8):
                    proj_add(0, cpt, pjT0[cpt])
                ln2_half(0)
                ffn_h1(0, w1pa, w1g0=w1g0)
                proj_half(1)
            ctxC.close()
            ctxW.close()
            if DEBUG_DUMP:
                for i in range(NTH):
                    nc.sync.dma_start(dbg_x2[128 * i : 128 * i + 128, :], x2[i][:])
            with (
                tc.tile_pool(name="w1pb", bufs=2, side="right") as w1pb,
                tc.tile_pool(name="w2p", bufs=8, side="right") as w2p,
            ):
                # h2(0) only needs half-0 state, so it runs first and its PE
                # stream covers ln2_half(1)'s ACT/DVE latency
                ffn_h2(0, w2p)
                ln2_half(1)
                ffn_h1(1, w1pb)
                ffn_h2(1, w2p)

    nc.compile()
    return nc


def _get_program():
    global _PROGRAM
    if _PROGRAM is None:
        _PROGRAM = _build_program()
    return _PROGRAM


def make_in_maps(x, Wq, Wk, Wv, Wproj, bproj, ln1_g, ln1_b, ln2_g, ln2_b, W1, b1, W2, b2):
    """Host-side sharding: build the 8 per-core input maps."""
    x = np.asarray(x, np.float32)
    Wq = np.asarray(Wq, np.float32)
    Wk = np.asarray(Wk, np.float32)
    Wv = np.asarray(Wv, np.float32)
    Wproj = np.asarray(Wproj, np.float32)
    bproj = np.asarray(bproj, np.float32)
    ln1_g = np.asarray(ln1_g, np.float32)
    ln1_b = np.asarray(ln1_b, np.float32)
    ln2_g = np.asarray(ln2_g, np.float32)
    ln2_b = np.asarray(ln2_b, np.float32)
    W1 = np.asarray(W1, np.float32)
    b1 = np.asarray(b1, np.float32)
    W2 = np.asarray(W2, np.float32)
    b2 = np.asarray(b2, np.float32)

    assert np.all(ln1_b == 0.0) and np.all(ln2_b == 0.0), (
        "nonzero LN bias folding not implemented"
    )

    scale = 1.0 / np.sqrt(C)
    # [H, C, HS] -> g-folded, concat to [C, H*HS]
    Wq_f = (ln1_g[None, :, None] * Wq * scale).transpose(1, 0, 2).reshape(C, H * HS)
    Wk_f = (ln1_g[None, :, None] * Wk).transpose(1, 0, 2).reshape(C, H * HS)
    Wv_f = (ln1_g[None, :, None] * Wv).transpose(1, 0, 2).reshape(C, H * HS)
    W1_f = ln2_g[:, None] * W1

    # causal binary mask for the [128,128] diagonal triangle, both heads
    tri = np.zeros((128, 2, 128), np.float32)
    s_idx = np.arange(128)[:, None]
    t_idx = np.arange(128)[None, :]
    tri[:, 0, :] = (s_idx <= t_idx).astype(np.float32)
    tri[:, 1, :] = tri[:, 0, :]

    # W1 device-read order: per mg-group [p, c, m] with source row c*128+p,
    # col mg*512+m  ->  host layout [mg, p, c, m]
    w1_blob = (
        W1_f.astype(BF16_NP)
        .reshape(NC8, 128, NM // 4, 512)
        .transpose(2, 1, 0, 3)
        .ravel()
    )
    # W2 device-read order: per (cp, q) block [p, j, c2] with source row
    # 512q+128j+p, col 512cp+c2  ->  host layout [cp, q, p, j, c2]
    w2_blob = (
        W2.astype(BF16_NP)
        .reshape(NM // 4, 4, 128, 2, 512)
        .transpose(3, 0, 2, 1, 4)
        .ravel()
    )
    b1c_blob = b1.reshape(FF // 128, 128).T.astype(np.float32).ravel()

    in_maps = []
    for c in range(N_CORES):
        b = c // 2
        hg = c % 2
        cols = slice(hg * HL * HS, (hg + 1) * HL * HS)
        # Wproj rows permuted: own head block first, then peer's
        own = Wproj[hg * HL * HS : (hg + 1) * HL * HS, :]
        peer = Wproj[(1 - hg) * HL * HS : (2 - hg) * HL * HS, :]
        selc = np.zeros((128, 2), np.float32)
        selc[:, 0] = hg
        selc[:, 1] = 1 - hg

        blob_h = np.zeros(BF_TOTAL, BF16_NP)
        blob_h[X_OFF : X_OFF + T * C] = x[b].astype(BF16_NP).ravel()
        blob_h[WQ_OFF : WQ_OFF + C * HL * HS] = Wq_f[:, cols].astype(BF16_NP).ravel()
        blob_h[WK_OFF : WK_OFF + C * HL * HS] = Wk_f[:, cols].astype(BF16_NP).ravel()
        blob_h[WV_OFF : WV_OFF + C * HL * HS] = Wv_f[:, cols].astype(BF16_NP).ravel()
        blob_h[WP_OFF : WP_OFF + C * C] = (
            np.concatenate([own, peer], axis=0).astype(BF16_NP).ravel()
        )
        blob_h[W1_OFF : W1_OFF + C * FF] = w1_blob
        blob_h[W2_OFF : W2_OFF + FF * C] = w2_blob
        blob_h[TRI_OFF : TRI_OFF + 128 * 2 * 128] = tri.astype(BF16_NP).ravel()
        blob_h[B2R_OFF : B2R_OFF + C] = b2.astype(BF16_NP).ravel()
        blob_h[ONES_OFF : ONES_OFF + 128] = np.ones(128, BF16_NP)

        blob_f = np.zeros(F32_TOTAL, np.float32)
        blob_f[XO_OFF : XO_OFF + TH * C] = (
            x[b, hg * TH : (hg + 1) * TH, :] + bproj[None, :]
        ).astype(np.float32).ravel()
        blob_f[B1C_OFF : B1C_OFF + 128 * NM] = b1c_blob
        blob_f[SELC_OFF : SELC_OFF + 256] = selc.ravel()

        in_maps.append({"blob_h": blob_h, "blob_f": blob_f})
    return in_maps


def assemble(results):
    out = np.empty((B, T, C), np.float32)
    for c in range(N_CORES):
        b, hg = c // 2, c % 2
        out[b, hg * TH : (hg + 1) * TH, :] = results[c]["out_half"]
    return out


def kernel(**inputs):
    from concourse import bass2jax

    nc = _get_program()
    in_maps = make_in_maps(**inputs)
    results = bass2jax.run_bass_via_pjrt(nc, in_maps, n_cores=N_CORES)
    return assemble(results)

